# revision 37
# baseline (speedup 1.0000x reference)
"""Trainium2 Bass kernel for the AUV Fossen dynamics RK2 step (nn_AUVFossen).

Per row (batch K): x[13] = pos(3)+quat(x,y,z,w)(4)+v(6), u[6].
  k1 = f(x,u); k2 = f(x + DT*k1, u); out = normquat(x + DT/2*(k1+k2))

Sharding: pure data parallel over 8 NeuronCores (batch split).
Layout: batch-major [128, C*W] tiles (W rows/partition, comps interleaved),
planarized to [slot, W] fp16 planes on ACT, computed on DVE + ACT.

DVE cost model (measured): tensor_tensor 2-byte unit-stride = 2x mode
(133ns per 256-elem slot), tensor_scalar/tensor_copy = 4x (67ns),
scalar_tensor_tensor = always 1x (267ns), fp32 = no 2x_1p. So the
kernel avoids stt (splits into ts+tt with scale folding: monomials
prescaled by 2 via q2 / sqrt2-squares, -m1 folded into the cross
products, linear damping folded via ts adds).

Precision: input quats are NOT normalized (|q|^2 up to ~30), so the
restoring terms reach ~5e5 and rhs2 ~1e8; dot1's rhs error is
amplified ~0.05x-abs into the output via dot2's quadratic damping.
fp16 (2.4e-4 rel) is required over bf16 (2e-3). The whole RHS path
runs in fp16 scaled by 1/S_RHS (folded into u/ld/qd/c1/c2/m1 and the
minv integrator scales) to stay in fp16 range; dot2's rot-apply block
(values up to ~2e5 in the worst-case envelope) runs bf16; the norm
reciprocal and final v accumulation are fp32.

Engine-boundary scheduling: tile-granular cross-engine deps mean the
first DVE op waits for every earlier ACT write to any tile it touches,
so ACT-written scratch (AB/QD4/pos/u/va_m) lives in its own tile (SCH)
and AB/QD4 are emitted after the S block, keeping the DVE start gated
only by the q/v planarize quarters.
"""

import os
import sys

for _p in ("/opt/trn_rl_repo", "/root/.axon_site/_ro/trn_rl_repo"):
    if os.path.isdir(_p) and _p not in sys.path:
        sys.path.insert(0, _p)

import numpy as np

import concourse.bacc as bacc
import concourse.bass as bass
import concourse.mybir as mybir
from concourse.alu_op_type import AluOpType
from concourse.tile import TileContext

F32 = mybir.dt.float32
F16 = mybir.dt.float16
BF16 = mybir.dt.bfloat16
AF = mybir.ActivationFunctionType
MULT = AluOpType.mult
ADD = AluOpType.add
SUB = AluOpType.subtract

DT = 0.1
GRAVITY = 9.81
DENSITY = 1028.0
S_RHS = 8192.0     # fp16 range scale for the RHS path

N_CORES = 8
P = 128
NSF = 10          # fp32 scratch slots
NSB = 84          # fp16 DVE scratch slots
NSH = 22          # fp16 ACT-written scratch slots (AB/QD4 shared by dots)
NSC = 16          # bf16 scratch slots (dot2 rot block)
NSG = 10          # fp16 scratch slots (cross products)

TRACE = False          # set by test.py
LAST_RUN_INFO = {}


class CompView:
    """Component view of a [128, C*W] interleaved tile (addr = w*C + c)."""

    def __init__(self, tile_ap, wstride, base, W):
        self.t = tile_ap.tensor
        self.off = tile_ap.offset + base
        self.part = tile_ap.ap[0]
        self.ws = wstride
        self.W = W

    def ap(self, c0, n=1, cstep=1, w0=0, wn=None):
        wn = self.W if wn is None else wn
        off = self.off + c0 + w0 * self.ws
        if n == 1:
            return bass.AP(self.t, off, [self.part, [self.ws, wn]])
        return bass.AP(self.t, off,
                       [self.part, [cstep, n], [self.ws, wn]])

    def bcast(self, c0, n):
        return bass.AP(self.t, self.off + c0,
                       [self.part, [0, n], [self.ws, self.W]])


class Region:
    """Contiguous run of planar scratch slots (slot = [128, W] plane)."""

    def __init__(self, scr, slot0, n):
        self.scr = scr
        self.slot0 = slot0
        self.n = n

    def ap(self, s0=0, n=1, sstep=1, w0=0, wn=None):
        scr = self.scr
        wn = scr.W if wn is None else wn
        off = scr.off + (self.slot0 + s0) * scr.pitch + w0
        if n == 1:
            return bass.AP(scr.t, off, [scr.part, [1, wn]])
        return bass.AP(scr.t, off,
                       [scr.part, [sstep * scr.pitch, n], [1, wn]])

    def bcast(self, s0, n):
        scr = self.scr
        off = scr.off + (self.slot0 + s0) * scr.pitch
        return bass.AP(scr.t, off, [scr.part, [0, n], [1, scr.W]])


class Scratch:
    def __init__(self, tile_ap, nslots, W, pitch=None):
        self.t = tile_ap.tensor
        self.off = tile_ap.offset
        self.part = tile_ap.ap[0]
        self.W = W
        self.pitch = W if pitch is None else pitch
        self.free_slots = set(range(nslots))
        self.regions = {}

    def alloc(self, name, n, high=False):
        fs = sorted(self.free_slots)
        run = None
        idxs = range(len(fs) - n + 1)
        if high:
            idxs = reversed(list(idxs))
        for i in idxs:
            if fs[i + n - 1] - fs[i] == n - 1:
                run = fs[i]
                break
        assert run is not None, f"scratch OOM for {name}({n}); free={len(fs)}"
        for s in range(run, run + n):
            self.free_slots.remove(s)
        self.regions[name] = (run, n)
        return Region(self, run, n)

    def free(self, *names):
        for name in names:
            run, n = self.regions.pop(name)
            self.free_slots.update(range(run, run + n))


def _extract_params(inputs):
    mass = float(np.asarray(inputs["mass"]).reshape(-1)[0])
    volume = float(np.asarray(inputs["volume"]).reshape(-1)[0])
    cog = np.asarray(inputs["cog"], np.float64).reshape(3)
    cob = np.asarray(inputs["cob"], np.float64).reshape(3)
    mTot = np.asarray(inputs["mTot"], np.float64).reshape(6, 6)
    linDamp = np.asarray(inputs["linDamp"], np.float64).reshape(6, 6)
    linDampFow = np.asarray(inputs["linDampFow"], np.float64).reshape(6, 6)
    quadDamp = np.asarray(inputs["quadDamp"], np.float64).reshape(6, 6)

    scale = max(np.abs(mTot).max(), 1e-30)
    tl, tr = mTot[0:3, 0:3], mTot[0:3, 3:6]
    bl, br = mTot[3:6, 0:3], mTot[3:6, 3:6]
    m1 = float(np.trace(tl) / 3.0)
    m2 = float(np.trace(br) / 3.0)
    structured = (
        np.abs(tl - m1 * np.eye(3)).max() < 1e-5 * scale
        and np.abs(br - m2 * np.eye(3)).max() < 1e-5 * scale
        and np.abs(tr).max() < 1e-5 * scale
        and np.abs(bl).max() < 1e-5 * scale
    )
    if not structured:
        raise NotImplementedError("unstructured mTot not supported")
    if np.abs(linDampFow).max() > 1e-30:
        raise NotImplementedError("nonzero linDampFow not supported")

    minv = np.diag(np.linalg.inv(mTot))
    ld = linDamp.copy()                 # -Dv_lin = +linDamp @ v
    qd = np.diag(quadDamp)              # only diag of quadDamp matters
    c1 = GRAVITY * (volume * DENSITY - mass)
    c2 = -mass * GRAVITY * cog + volume * DENSITY * GRAVITY * cob
    return dict(m1=m1, minv=minv, ld=ld, qd=qd, c1=float(c1), c2=c2)


def _val_runs(vals, nonzero_only=False):
    """Group consecutive equal values: [(i0, n, val)]."""
    out = []
    for i, v in enumerate(vals):
        if nonzero_only and v == 0.0:
            continue
        if out and out[-1][2] == v and i == out[-1][0] + out[-1][1]:
            out[-1] = (out[-1][0], out[-1][1] + 1, v)
        else:
            out.append((i, 1, v))
    return out


def build_program(pp, K_core, W):
    # Uneven chunk split: a small first chunk lands its X DMA (and so the
    # planarize and the DVE start) ~2x sooner; total instruction count and
    # slot work are invariant to the split, so the ramp saving is free.
    chunk_ws = [W, W]
    assert K_core == P * sum(chunk_ws)
    n_chunks = len(chunk_ws)
    WP = max(chunk_ws)               # scratch pitch (scr tiles are shared)
    r0s = [P * sum(chunk_ws[:i]) for i in range(n_chunks)]

    nc = bacc.Bacc("TRN2", target_bir_lowering=False, debug=False,
                   num_devices=N_CORES)
    x_d = nc.dram_tensor("x", (K_core, 13), F32, kind="ExternalInput")
    u_d = nc.dram_tensor("u", (K_core, 6), F32, kind="ExternalInput")
    o_d = nc.dram_tensor("o", (K_core, 13), F32, kind="ExternalOutput")
    xt = x_d[:, :].tensor
    ut = u_d[:, :].tensor
    ot = o_d[:, :].tensor

    def dram_ap(t, ncomp, ci, c0, cn):
        # [p, (w c)] view of chunk ci rows (p-major), free cols [c0, c0+cn)
        Wc = chunk_ws[ci]
        return bass.AP(t, r0s[ci] * ncomp + c0,
                       [[ncomp * Wc, P], [1, cn]])

    with TileContext(nc) as tc:
        with tc.tile_pool(name="io", bufs=1) as iop, \
             tc.tile_pool(name="pv", bufs=1) as pvp, \
             tc.tile_pool(name="scr", bufs=1) as scrp:
            tiles = []
            SCF = scrp.tile([P, NSF * WP], F32, tag="SCF")
            SCB = scrp.tile([P, NSB * WP], F16, tag="SCB")
            SCC = scrp.tile([P, NSC * WP], BF16, tag="SCC")
            SCG = scrp.tile([P, NSG * WP], F16, tag="SCG")
            for ci, Wc in enumerate(chunk_ws):
                X = iop.tile([P, 13 * Wc], F32, tag=f"X{ci}")
                U = iop.tile([P, 6 * Wc], F32, tag=f"U{ci}")
                O = iop.tile([P, 13 * Wc], F32, tag=f"O{ci}")
                XB = pvp.tile([P, 10 * Wc], F16, tag=f"XB{ci}")
                SCH = pvp.tile([P, NSH * Wc], F16, tag=f"SCH{ci}")
                tiles.append((X, U, O, XB, SCH, SCF, SCB, SCC, SCG))

            def x_dmas(ci):
                # X lands in four w-quarters so the planarize (and with it
                # the DVE) can start as soon as the first quarter arrives.
                X = tiles[ci][0]
                qw = 13 * (chunk_ws[ci] // 4)
                for k in range(4):
                    nc.sync.dma_start(X[:, k * qw:(k + 1) * qw],
                                      dram_ap(xt, 13, ci, k * qw, qw))

            def u_dma(ci):
                nc.sync.dma_start(tiles[ci][1][:, :],
                                  dram_ap(ut, 6, ci, 0, 6 * chunk_ws[ci]))

            # DMA queues are FIFO: everything enqueued up front steals
            # bandwidth from chunk0's X, which gates the DVE start. Token
            # writes (ACT copies into the target tiles) delay chunk0's U
            # until X0 has landed and chunk1's inputs until the chunk0
            # planarize is done, giving X0 exclusive bandwidth.
            a0 = nc.scalar
            x_dmas(0)
            emit_planarize(nc, tiles[0], chunk_ws[0])
            X0, XB0 = tiles[0][0], tiles[0][3]
            a0.activation(tiles[0][1][:, 0:1], X0[:, 0:1], AF.Copy)
            u_dma(0)
            for ci in range(1, n_chunks):
                a0.activation(tiles[ci][0][:, 0:1], XB0[:, 0:1], AF.Copy)
                a0.activation(tiles[ci][1][:, 0:1], XB0[:, 0:1], AF.Copy)
                x_dmas(ci)
                u_dma(ci)
            for ci, Wc in enumerate(chunk_ws):
                O = tiles[ci][2]
                last = ci == n_chunks - 1
                plan_next = None
                if not last:
                    nxt = tiles[ci + 1]
                    wn_ = chunk_ws[ci + 1]
                    plan_next = (lambda nx=nxt, w=wn_:
                                 emit_planarize(nc, nx, w))
                tail_dmas = []
                emit_chunk(nc, pp, tiles[ci], Wc, WP,
                           split_tail=last, tail_dmas=tail_dmas,
                           plan_next=plan_next, first_chunk=(ci == 0))
                for w0, wn in tail_dmas:
                    nc.sync.dma_start(dram_ap(ot, 13, ci, 13 * w0, 13 * wn),
                                      O[:, 13 * w0:13 * (w0 + wn)])
    nc.compile()
    return nc


# XB fp16 planar layout (10 slots): 0:4 q (x,y,z,w), 4:10 v (vl 4:7, va 7:10)
# SCH fp16 (ACT-written scratch): 0:3 pos, 3:9 u' (=u/S), 9:12 va_m1,
#   12:18 AB1, 18:22 QD4_1, 22:28 AB2, 28:32 QD4_2
def emit_planarize(nc, tiles, W):
    a = nc.scalar
    Xt = tiles[0][:, :]
    XBt = tiles[3][:, :]
    xall = CompView(Xt, 13, 0, W)
    XBR = Region(Scratch(XBt, 10, W), 0, 10)
    qw = W // 4
    for k in range(4):
        a.activation(XBR.ap(0, 10, w0=k * qw, wn=qw),
                     xall.ap(3, 10, w0=k * qw, wn=qw), AF.Copy)


def emit_chunk(nc, pp, tiles, W, WP, split_tail=False, tail_dmas=None,
               plan_next=None, first_chunk=False):
    v = nc.vector
    a = nc.scalar
    Xt, Ut, Ot, XBt, SCHt, SCFt, SCBt, SCCt, SCGt = (t[:, :] for t in tiles)
    scrf = Scratch(SCFt, NSF, W, pitch=WP)
    scrb = Scratch(SCBt, NSB, W, pitch=WP)
    scrc = Scratch(SCCt, NSC, W, pitch=WP)
    scg = Scratch(SCGt, NSG, W, pitch=WP)
    sch = Scratch(SCHt, NSH, W)
    xb = Scratch(XBt, 10, W)
    mv = pp["minv"]
    m1s = pp["m1"] / S_RHS

    oint = CompView(Ot, 13, 0, W)
    uall = CompView(Ut, 6, 0, W)
    xall = CompView(Xt, 13, 0, W)

    xq = Region(xb, 0, 4)
    vl1 = Region(xb, 4, 3)
    va1 = Region(xb, 7, 3)
    v61 = Region(xb, 4, 6)
    xpos = Region(sch, 0, 3)
    UF = Region(sch, 3, 6)
    vam1 = Region(sch, 9, 3)
    AB1 = Region(sch, 12, 6)
    QD41 = Region(sch, 18, 4)
    AB2 = AB1      # reused; dot2's ACT writes serialize behind dot1's reads
    QD42 = QD41

    OPL = scrb.alloc("OPL", 7)    # [pos(0:3) quat(3:7)]
    S1R = scrb.alloc("S1R", 4)
    S2R = scrb.alloc("S2R", 4)
    PL1 = scrb.alloc("PL1", 3)
    PL2 = scrc.alloc("PL2", 3)
    Q2 = scrb.alloc("Q2", 4)
    V2 = scrb.alloc("V2", 6)
    VAM2 = scrb.alloc("VAM2", 3)
    RHS1 = scrb.alloc("RHS1", 6)
    RHS2 = scrb.alloc("RHS2", 6)

    def act_feeders1():
        # SCH writes only — never blocks the DVE's SCB/XB stream.
        a.activation(xpos.ap(0, 3), xall.ap(0, 3), AF.Copy)
        a.activation(UF.ap(0, 6), uall.ap(0, 6), AF.Copy,
                     scale=float(1.0 / S_RHS))
        a.activation(vam1.ap(0, 3), xall.ap(10, 3), AF.Copy,
                     scale=float(-m1s))

    emit_dot(nc, pp, scrf, scrb, scrb, scg, "d1", xq, vl1, va1, v61,
             vam1, UF, AB1, QD41, S1R, RHS1, PL1, act_pre=act_feeders1,
             s_halves=False)

    # x2 = x + DT*k1  (S1R = 2*pDot_ang with S[0] sign-flipped)
    TQ = scrb.alloc("TQ", 4)
    v.tensor_scalar(TQ.ap(0), S1R.ap(0), -DT / 2, None, MULT)
    v.tensor_scalar(TQ.ap(1, 3), S1R.ap(1, 3), DT / 2, None, MULT)
    v.tensor_tensor(Q2.ap(0, 4), TQ.ap(0, 4), xq.ap(0, 4), ADD)
    scrb.free("TQ")
    TV = scrb.alloc("TV", 6)
    for i0, n, mval in _val_runs(mv):
        v.tensor_scalar(TV.ap(i0, n), RHS1.ap(i0, n),
                        DT * mval * S_RHS, None, MULT)
    v.tensor_tensor(V2.ap(0, 6), TV.ap(0, 6), v61.ap(0, 6), ADD)
    scrb.free("TV")
    v.tensor_scalar(VAM2.ap(0, 3), V2.ap(3, 3), float(-m1s), None, MULT)

    # emit the next chunk's planarize here (not after dot2): ACT must get
    # through it plus dot2's feeders and the deplanarize before this
    # chunk's output adds, or the DVE stalls on the O-tile WAW.
    if plan_next is not None:
        plan_next()
        plan_next = None

    norm_regs = {}

    def _norm_chain():
        SS = scrb.alloc("SS", 4, high=True)
        QR = scrb.alloc("QR", 4, high=True)
        TQR = scrb.alloc("TQR", 4, high=True)
        v.tensor_tensor(SS.ap(0, 4), S1R.ap(0, 4), S2R.ap(0, 4), ADD)
        v.tensor_scalar(TQR.ap(0), SS.ap(0), -DT / 4, None, MULT)
        v.tensor_scalar(TQR.ap(1, 3), SS.ap(1, 3), DT / 4, None, MULT)
        v.tensor_tensor(QR.ap(0, 4), TQR.ap(0, 4), xq.ap(0, 4), ADD)
        scrb.free("SS", "TQR")
        NQ = scrf.alloc("NQ", 4, high=True)
        NS2 = scrf.alloc("NS2", 2, high=True)
        NS1 = scrf.alloc("NS1", 1, high=True)
        SQC = scrf.alloc("SQC", 1, high=True)
        RINV = scrf.alloc("RINV", 1, high=True)
        RINVB = scrb.alloc("RINVB", 1, high=True)
        a.activation(NQ.ap(0, 4), QR.ap(0, 4), AF.Square)
        v.tensor_tensor(NS2.ap(0, 2), NQ.ap(0, 2), NQ.ap(2, 2), ADD)
        v.tensor_tensor(NS1.ap(0), NS2.ap(0), NS2.ap(1), ADD)
        a.activation(SQC.ap(0), NS1.ap(0), AF.Sqrt)
        v.reciprocal_approx_fast(RINV.ap(0), SQC.ap(0))
        v.tensor_copy(RINVB.ap(0), RINV.ap(0))
        scrf.free("NQ", "NS2", "NS1", "SQC", "RINV")
        norm_regs["QR"] = QR
        norm_regs["RINVB"] = RINVB

    emit_dot(nc, pp, scrf, scrb, scrc, scg, "d2", Q2,
             Region(scrb, V2.slot0, 3), Region(scrb, V2.slot0 + 3, 3),
             V2, VAM2, UF, AB2, QD42, S2R, RHS2, PL2,
             post_s=_norm_chain, rhs_base=RHS1)

    # ---- outputs ----
    QR, RINVB = norm_regs["QR"], norm_regs["RINVB"]
    v.tensor_tensor(OPL.ap(3, 4), QR.ap(0, 4), RINVB.bcast(0, 4), MULT)
    scrb.free("QR", "RINVB", "S1R", "S2R")

    TMP3 = scrc.alloc("TMP3", 3)
    v.tensor_tensor(TMP3.ap(0, 3), PL1.ap(0, 3), PL2.ap(0, 3), ADD)
    v.tensor_scalar(TMP3.ap(0, 3), TMP3.ap(0, 3), DT / 2, None, MULT)
    v.tensor_tensor(OPL.ap(0, 3), TMP3.ap(0, 3), xpos.ap(0, 3), ADD)
    scrc.free("TMP3", "PL2")
    scrb.free("PL1")

    # RHS2 already holds RHS1 + RHS2 (rhs_base fold in dot 2).
    # pos+quat deplanarize on ACT; v deplanarize fused into the final
    # fp32 add on DVE (strided dst is 1x anyway).
    TVO = scrf.alloc("TVO", 6)
    for i0, n, mval in _val_runs(mv):
        v.tensor_scalar(TVO.ap(i0, n), RHS2.ap(i0, n),
                        DT / 2 * mval * S_RHS, None, MULT)
    # decreasing piece sizes: the last piece's out-DMA (which trails the
    # final DVE op) is the shortest
    e = W // 8
    pieces = ((0, 4 * e), (4 * e, 2 * e), (6 * e, e), (7 * e, e))
    for w0, wn in pieces:
        a.activation(oint.ap(0, 7, w0=w0, wn=wn),
                     OPL.ap(0, 7, w0=w0, wn=wn), AF.Copy)
    for w0, wn in pieces:
        v.tensor_tensor(oint.ap(7, 6, w0=w0, wn=wn),
                        TVO.ap(0, 6, w0=w0, wn=wn),
                        v61.ap(0, 6, w0=w0, wn=wn), ADD)
        tail_dmas.append((w0, wn))
    scrf.free("TVO")
    scrb.free("OPL", "RHS1", "RHS2", "Q2", "V2", "VAM2")


def emit_dot(nc, pp, scrf, scrb, scrr, scg, tag, q, vl, va, v6, va_m, UF,
             AB, QD4, S, RHS, PL, act_pre=None, post_s=None, rhs_base=None,
             s_halves=False):
    """One f() evaluation.
    q/vl/va/v6: fp16 planar Regions (v6 = vl++va contiguous);
    va_m: -(m1/S_RHS)*va fp16; UF: u/S_RHS fp16; AB/QD4: SCH regions
    (ACT-written); scrr: scratch for the rot-apply block (fp16 for dot1,
    bf16 for dot2 whose R*vl products can exceed fp16 range).
    Outputs: S[4] (2*pDot_ang, S[0] sign-flipped), RHS[6] fp16 scaled by
    1/S_RHS, PL[3] (pDot_lin).
    """
    v = nc.vector
    a = nc.scalar
    m1, ld, qd, c1, c2 = pp["m1"], pp["ld"], pp["qd"], pp["c1"], pp["c2"]
    c1s = c1 / S_RHS
    c2s = c2 / S_RHS
    qds = qd / S_RHS
    lds = np.diag(ld) / S_RHS

    # S rows first: the only DVE deps are XB/V2, so the chunk's DVE stream
    # starts as soon as the planarize lands. Products are j-major
    # (TP[4j+i] = q_i * w_j, one va-broadcast instr per w comp):
    #   S0' =  x*w0 + y*w1 + z*w2 = TP0 + TP5 + TP10
    #   S1  =  w*w0 - z*w1 + y*w2 = TP3 + TP9 - TP6
    #   S2  =  z*w0 + w*w1 - x*w2 = TP2 + TP7 - TP8
    #   S3  = -y*w0 + x*w1 + w*w2 = TP4 + TP11 - TP1
    TP = scrb.alloc("TP", 12)
    AS = scrb.alloc("AS", 4)
    for j in range(3):
        v.tensor_tensor(TP.ap(4 * j, 4), q.ap(0, 4), va.bcast(j, 4), MULT)
    v.tensor_tensor(AS.ap(0, 2), TP.ap(0, 2, 3), TP.ap(5, 2, 4), ADD)
    v.tensor_tensor(AS.ap(2, 2), TP.ap(2, 2, 2), TP.ap(7, 2, 4), ADD)
    v.tensor_tensor(S.ap(0), AS.ap(0), TP.ap(10), ADD)
    v.tensor_tensor(S.ap(1, 2), AS.ap(1, 2), TP.ap(6, 2, 2), SUB)
    v.tensor_tensor(S.ap(3), AS.ap(3), TP.ap(1), SUB)
    scrb.free("TP", "AS")

    # ACT feeders (SCH tile): AB_i = |(qd_i/S) * v_i|, doubled squares.
    if act_pre is not None:
        act_pre()
    for i in range(6):
        a.activation(AB.ap(i), v6.ap(i), AF.Abs, scale=float(qds[i]))
    SQRT2 = float(np.sqrt(2.0))
    a.activation(QD4.ap(0, 2), q.ap(1, 2), AF.Square, scale=SQRT2)
    a.activation(QD4.ap(2, 2), q.ap(0, 2), AF.Square, scale=SQRT2)

    # coriolis: CR = -(m1/S)*(va x vl) via the prescaled va_m.
    # (GPSIMD was tried for this block and slowed the DVE ~10% via SBUF
    # port contention — keep it on the DVE.)
    PA = scg.alloc(f"PA{tag}", 3)
    PB = scg.alloc(f"PB{tag}", 3)
    CR = scg.alloc(f"CR{tag}", 3)
    v.tensor_tensor(PA.ap(0, 2), va_m.ap(1, 2), vl.ap(2, 2, -2), MULT)
    v.tensor_tensor(PA.ap(2), va_m.ap(0), vl.ap(1), MULT)
    v.tensor_tensor(PB.ap(1, 2), va_m.ap(0, 2), vl.ap(2, 2, -2), MULT)
    v.tensor_tensor(PB.ap(0), va_m.ap(2), vl.ap(1), MULT)
    v.tensor_tensor(CR.ap(0, 3), PA.ap(0, 3), PB.ap(0, 3), SUB)

    if post_s is not None:
        post_s()

    # doubled quat: q2 = 2q (feeds the doubled monomials)
    Q2D = scrb.alloc("Q2D", 4)
    v.tensor_scalar(Q2D.ap(0, 4), q.ap(0, 4), 2.0, None, MULT)

    # doubled monomials: P1 = [2xy, 2xz, 2yz], P2 = [2zw, 2yw, 2xw]
    P1 = scrb.alloc("P1", 3)
    P2 = scrb.alloc("P2", 3)
    v.tensor_tensor(P1.ap(0, 2), Q2D.bcast(0, 2), q.ap(1, 2), MULT)
    v.tensor_tensor(P1.ap(2), Q2D.ap(1), q.ap(2), MULT)
    v.tensor_tensor(P2.ap(0, 2), q.bcast(3, 2), Q2D.ap(2, 2, -1), MULT)
    v.tensor_tensor(P2.ap(2), Q2D.ap(0), q.ap(3), MULT)
    scrb.free("Q2D")

    # QO = 2*[Qo10, Qo02, Qo21, Qo01, Qo20, Qo12]; QDG = 2*(yy+zz, zz+xx, xx+yy)
    QO = scrb.alloc("QO", 6)
    QDG = scrb.alloc("QDG", 3)
    v.tensor_tensor(QO.ap(0, 3), P1.ap(0, 3), P2.ap(0, 3), ADD)
    v.tensor_tensor(QO.ap(3, 3), P1.ap(0, 3), P2.ap(0, 3), SUB)
    v.tensor_tensor(QDG.ap(0, 3), QD4.ap(0, 3), QD4.ap(1, 3), ADD)
    scrb.free("P1", "P2")

    # pDot_lin = vl + (2Q @ vl)  (the 2 is folded into QO/QDG)
    RD = scrr.alloc("RD", 3)
    RO = scrr.alloc("RO", 6)     # 2*[R01, R02, R10, R12, R20, R21]
    T1 = scrr.alloc("T1", 3)
    v.tensor_tensor(RD.ap(0, 3), QDG.ap(0, 3), vl.ap(0, 3), MULT)
    v.tensor_tensor(RO.ap(2, 2, -1), QO.ap(0, 2), vl.ap(0, 2, 2), MULT)
    v.tensor_tensor(RO.ap(0, 2, 4), QO.ap(3, 2), vl.ap(1, 2, -1), MULT)
    v.tensor_tensor(RO.ap(5), QO.ap(2), vl.ap(1), MULT)           # R21
    v.tensor_tensor(RO.ap(3), QO.ap(5), vl.ap(2), MULT)           # R12
    v.tensor_tensor(T1.ap(0, 3), RO.ap(0, 3, 2), RO.ap(1, 3, 2), ADD)
    v.tensor_tensor(T1.ap(0, 3), T1.ap(0, 3), RD.ap(0, 3), SUB)
    v.tensor_tensor(PL.ap(0, 3), T1.ap(0, 3), vl.ap(0, 3), ADD)
    scrr.free("RD", "RO", "T1")

    # damping: ABL_i = |qd_i v_i|/S - ld_ii/S  (>=0 since ld<0, qd<0);
    # then T6 = ABL * v and RHS = u/S - T6 = (u + (ld + qd|v|) v)/S.
    ABL = scrb.alloc("ABL", 6)
    T6 = scrb.alloc("T6", 6)
    assert all(qd[i] <= 0 for i in range(6)), "positive quadDamp unsupported"
    for i0, n, lv in _val_runs([-lds[i] for i in range(6)]):
        v.tensor_scalar(ABL.ap(i0, n), AB.ap(i0, n), float(lv), None, ADD)
    v.tensor_tensor(T6.ap(0, 6), ABL.ap(0, 6), v6.ap(0, 6), MULT)
    scrb.free("ABL")

    # rhs = (u + (ld + qd|v|) v - m1*(va x vl) + g-terms)/S
    v.tensor_tensor(RHS.ap(0, 6), UF.ap(0, 6), T6.ap(0, 6), SUB)
    scrb.free("T6")
    if rhs_base is not None:
        v.tensor_tensor(RHS.ap(0, 6), RHS.ap(0, 6), rhs_base.ap(0, 6), ADD)
    for i in range(6):
        for j in range(6):
            if i != j and ld[i, j] != 0.0:
                v.scalar_tensor_tensor(RHS.ap(i), v6.ap(j),
                                       float(ld[i, j] / S_RHS),
                                       RHS.ap(i), MULT, ADD)
    v.tensor_tensor(RHS.ap(0, 3), RHS.ap(0, 3), CR.ap(0, 3), ADD)
    scg.free(f"PA{tag}", f"PB{tag}", f"CR{tag}")

    # restoring: rot row2 = (2Qo20, 2Qo21, 1-2Qd2) = (QO[4], QO[2], 1-QDG[2])
    # rhs rows0:2 += c1*rot2/S; rows3:5 += (c2 x rot2)/S
    c2x, c2y, c2z = (float(c2s[0]), float(c2s[1]), float(c2s[2]))
    if c2x == 0.0 and c2y == 0.0:
        # fast path: one fused 5-slot add
        # GV = [c1*2Qo20, c1*2Qo21, c1*(1-2Qd2), -c2z*2Qo21, c2z*2Qo20]
        GV = scrb.alloc("GV", 5)
        v.tensor_scalar(GV.ap(0, 2), QO.ap(4, 2, -2), float(c1s), None, MULT)
        v.tensor_scalar(GV.ap(2), QDG.ap(2), float(-c1s), float(c1s),
                        MULT, ADD)
        if c2z != 0.0:
            v.tensor_scalar(GV.ap(3), QO.ap(2), float(-c2z), None, MULT)
            v.tensor_scalar(GV.ap(4), QO.ap(4), float(c2z), None, MULT)
            v.tensor_tensor(RHS.ap(0, 5), RHS.ap(0, 5), GV.ap(0, 5), ADD)
        else:
            v.tensor_tensor(RHS.ap(0, 3), RHS.ap(0, 3), GV.ap(0, 3), ADD)
        scrb.free("GV", "QO", "QDG")
    else:
        TMPG = scrb.alloc("TMPG", 2)
        QG = scrb.alloc("QG", 2)
        v.tensor_scalar(QG.ap(0, 2), QO.ap(4, 2, -2), float(c1s), None, MULT)
        v.tensor_tensor(RHS.ap(0, 2), RHS.ap(0, 2), QG.ap(0, 2), ADD)
        v.tensor_scalar(TMPG.ap(0), QDG.ap(2), float(-c1s), float(c1s),
                        MULT, ADD)
        v.tensor_tensor(RHS.ap(2), RHS.ap(2), TMPG.ap(0), ADD)
        scrb.free("QG")
        # row3 += -c2z*(2Qo21) + c2y*(1-2Qd2)
        # row4 += c2z*(2Qo20) - c2x*(1-2Qd2)
        # row5 += c2x*(2Qo21) - c2y*(2Qo20)
        if c2z != 0.0:
            QG2 = scrb.alloc("QG2", 2)
            v.tensor_scalar(QG2.ap(0, 2), QO.ap(2, 2, 2), float(c2z),
                            None, MULT)
            v.tensor_tensor(RHS.ap(3), RHS.ap(3), QG2.ap(0), SUB)
            v.tensor_tensor(RHS.ap(4), RHS.ap(4), QG2.ap(1), ADD)
            scrb.free("QG2")
        gterms = [
            (3, [(QDG, 2, -c2y)], c2y),
            (4, [(QDG, 2, c2x)], -c2x),
            (5, [(QO, 2, c2x), (QO, 4, -c2y)], 0.0),
        ]
        for row, terms, const in gterms:
            terms = [(reg, s, co) for (reg, s, co) in terms if co != 0.0]
            if const != 0.0:
                if terms:
                    reg, s, co = terms.pop(0)
                    v.tensor_scalar(TMPG.ap(1), reg.ap(s), co, const,
                                    MULT, ADD)
                    v.tensor_tensor(RHS.ap(row), RHS.ap(row), TMPG.ap(1), ADD)
                else:
                    v.tensor_scalar(RHS.ap(row), RHS.ap(row), const,
                                    None, ADD)
            for reg, s, co in terms:
                v.scalar_tensor_tensor(RHS.ap(row), reg.ap(s), co,
                                       RHS.ap(row), MULT, ADD)
        scrb.free("TMPG", "QO", "QDG")


_CACHE = {}


def kernel(**inputs):
    from concourse.bass_utils import run_bass_kernel_spmd

    x = np.ascontiguousarray(np.asarray(inputs["x"], np.float32))
    u = np.ascontiguousarray(np.asarray(inputs["u"], np.float32))
    K = x.shape[0]
    assert K % N_CORES == 0
    K_core = K // N_CORES
    W = 256
    assert K_core % (P * W) == 0

    pp = _extract_params(inputs)
    pp_key = (K_core, W, pp["m1"], pp["c1"], tuple(pp["minv"]),
              tuple(pp["qd"]), tuple(pp["c2"]), pp["ld"].tobytes())
    if pp_key not in _CACHE:
        _CACHE[pp_key] = build_program(pp, K_core, W)
    nc = _CACHE[pp_key]

    in_maps = []
    for k in range(N_CORES):
        sl = slice(k * K_core, (k + 1) * K_core)
        in_maps.append({"x": x[sl], "u": u[sl]})

    kwargs = dict(trace=True) if TRACE else {}
    res = run_bass_kernel_spmd(nc, in_maps, core_ids=list(range(N_CORES)),
                               **kwargs)
    LAST_RUN_INFO.clear()
    LAST_RUN_INFO.update(dict(
        exec_time_ns=res.exec_time_ns,
        mean_exec_time_ns=res.mean_exec_time_ns,
        profile_json=res.profile_json,
    ))
    out = np.empty((K, 13), np.float32)
    for k in range(N_CORES):
        out[k * K_core:(k + 1) * K_core] = res.results[k]["o"]
    return out
